# revision 4
# baseline (speedup 1.0000x reference)
"""Trainium2 Bass kernel for nn_AwkwardRNNDoubleJagged (8-core tensor-parallel LSTM).

Strategy
--------
The module is one long, strictly sequential LSTM chain: 64 particles, each a
ragged sequence of scalar inputs, with the event-level half-state carried
across particles.  Only sum(lengths) steps actually change state, so the host
flattens the valid steps into one schedule.

Per step the dominant work is the matvec W_hh @ h with W_hh [8192, 2048].
We shard the 4H gate dimension across the 8 NeuronCores (1024 rows/core,
bf16, SBUF-resident) and all-gather the bf16 hidden state (256 floats/core)
between steps via the ncfw AllGather collective.

Particle boundaries (h,c <- [second_half, 0]) are expressed as per-step mask
data (m=boundary, w=1-m), so every step runs the identical instruction
sequence and a single compiled program can execute any run of D consecutive
steps.  Since long chains of collectives in one NEFF are not reliable, the
chain is split into segments of D=64 steps executed back-to-back (state
stays device-resident between segments).

Hidden layout: h_all[p, 2q+e] = h[e*1024 + 128 q + p]; core m owns q = m.
Gate columns: [i0, f0, o0, i1, f1, o1, g0, g1] (Xe = gate X, hidden half e).
"""
import numpy as np
import ml_dtypes

NCORES = 8
H = 2048
SEG_D = 64
KERNEL_STATS = {}
GATE_OF_COL = [0, 1, 3, 0, 1, 3, 2, 2]
HALF_OF_COL = [0, 0, 0, 1, 1, 1, 0, 1]


def _host_prep(event, lengths, W_ih, W_hh, b_ih, b_hh):
    event = np.asarray(event, np.float32)
    lengths = np.asarray(lengths).astype(np.int64)
    W_hh = np.asarray(W_hh, np.float32)
    w_in = np.asarray(W_ih, np.float32)[:, 0]
    bsum = np.asarray(b_ih, np.float32) + np.asarray(b_hh, np.float32)

    xs, bnd = [], []
    for p in range(event.shape[0]):
        for t in range(int(lengths[p])):
            xs.append(event[p, t])
            bnd.append(1.0 if t == 0 else 0.0)
    xs = np.asarray(xs, np.float32)
    bnd = np.asarray(bnd, np.float32)
    S = len(xs)

    cols = np.arange(8)
    gates = np.asarray(GATE_OF_COL)[cols]
    halves = np.asarray(HALF_OF_COL)[cols]
    p_idx = np.arange(128)
    m_idx = np.arange(NCORES)
    rows = (gates[None, :, None] * 2048 + halves[None, :, None] * 1024
            + 128 * m_idx[:, None, None] + p_idx[None, None, :])  # [m, col, p]
    kc = np.arange(16)
    qs, es = kc // 2, kc % 2
    khid = es[:, None] * 1024 + 128 * qs[:, None] + np.arange(128)[None, :]

    Wt_cores, PS_cores = [], []
    for m in range(NCORES):
        g = W_hh[rows[m][:, None, None, :], khid[None, :, :, None]]
        g = np.transpose(g, (2, 0, 1, 3)).reshape(128, 8 * 16 * 128)
        Wt_cores.append(np.ascontiguousarray(g.astype(ml_dtypes.bfloat16)))
        r = rows[m]
        Bt = bsum[r][None] + w_in[r][None] * xs[:, None, None]   # [S, 8, 128]
        ps = np.zeros((S, 128, 10), np.float32)
        ps[:, :, 0:8] = np.transpose(Bt, (0, 2, 1))
        ps[:, :, 8] = bnd[:, None]
        ps[:, :, 9] = 1.0 - bnd[:, None]
        PS_cores.append(np.ascontiguousarray(ps))
    return S, Wt_cores, PS_cores


def _patch_birsim_off():
    """walrus's birsim pass simulates the whole program at compile time;
    for our ~10k-instruction segments that is minutes of compile for no
    benefit.  Rebuild bir_verify_and_optimise with birsim disabled."""
    import inspect
    import concourse.bass_utils as bu
    if getattr(bu, "_birsim_patched", False):
        return
    src = inspect.getsource(bu.bir_verify_and_optimise)
    src = src.replace('"--enable-birsim=true",', '"--enable-birsim=false",')
    exec(src, bu.__dict__)
    bu._birsim_patched = True


def _build_segment(D):
    import concourse.bass as bass
    import concourse.bacc as bacc
    import concourse.tile as tile
    import concourse.mybir as mybir
    _patch_birsim_off()
    F32 = mybir.dt.float32
    BF16 = mybir.dt.bfloat16
    AFT = mybir.ActivationFunctionType

    nc = bacc.Bacc("TRN2", target_bir_lowering=False, debug=False,
                   num_devices=NCORES)
    wt_dram = nc.dram_tensor("wt", [128, 8 * 16 * 128], BF16, kind="ExternalInput")
    ps_dram = nc.dram_tensor("perstep", [D, 128, 10], F32, kind="ExternalInput")
    hall_in = nc.dram_tensor("hall_in", [128, 16], BF16, kind="ExternalInput")
    c_in = nc.dram_tensor("c_in", [128, 2], F32, kind="ExternalInput")
    hall_out = nc.dram_tensor("hall_out", [128, 16], BF16, kind="ExternalOutput")
    c_out = nc.dram_tensor("c_out", [128, 2], F32, kind="ExternalOutput")
    h32_out = nc.dram_tensor("h32_out", [128, 2], F32, kind="ExternalOutput")

    with tile.TileContext(nc) as tc:
        with tc.tile_pool(name="wt", bufs=1) as wtp, \
             tc.tile_pool(name="state", bufs=1) as stp, \
             tc.tile_pool(name="psin", bufs=4) as psp_in, \
             tc.tile_pool(name="tmp", bufs=3) as tp, \
             tc.tile_pool(name="gps", bufs=2, space="PSUM") as psp, \
             tc.tile_pool(name="dram", bufs=2, space="DRAM") as dr:

            wt = wtp.tile([128, 8 * 16 * 128], BF16)
            nc.sync.dma_start(wt[:], wt_dram[:])
            h_all = stp.tile([128, 16], BF16)
            c = stp.tile([128, 2], F32)
            h32 = stp.tile([128, 2], F32)
            nc.sync.dma_start(h_all[:], hall_in[:])
            nc.sync.dma_start(c[:], c_in[:])

            ag_in = dr.tile([128, 2], BF16, tag="agin")
            ag_out = dr.tile([128 * NCORES, 2], BF16, tag="agout")

            def wtile(col, kcc):
                return wt[:, bass.ts(col * 16 + kcc, 128)]

            for s in range(D):
                ps = psp_in.tile([128, 10], F32, tag="ps")
                nc.sync.dma_start(ps[:], ps_dram[s])
                mm = ps[:, 8:9]
                ww = ps[:, 9:10]

                h_use = tp.tile([128, 16], BF16, tag="huse")
                nc.vector.tensor_scalar_mul(h_use[:], h_all[:], ww)
                nc.vector.scalar_tensor_tensor(
                    h_use[:, 0:16:2], h_all[:, 1:16:2], mm, h_use[:, 0:16:2],
                    op0=mybir.AluOpType.mult, op1=mybir.AluOpType.add)
                c_sel = tp.tile([128, 2], F32, tag="csel")
                nc.vector.tensor_scalar_mul(c_sel[:], c[:], ww)
                nc.vector.scalar_tensor_tensor(
                    c_sel[:, 0:1], c[:, 1:2], mm, c_sel[:, 0:1],
                    op0=mybir.AluOpType.mult, op1=mybir.AluOpType.add)

                psum = psp.tile([128, 8], F32, tag="gates")
                for col in range(8):
                    o = psum[:, col:col + 1]
                    for kcc in range(16):
                        nc.tensor.matmul(o, wtile(col, kcc),
                                         h_use[:, kcc:kcc + 1],
                                         start=(kcc == 0), stop=(kcc == 15))
                nc.vector.tensor_add(psum[:, 0:8], psum[:, 0:8], ps[:, 0:8])
                sg = tp.tile([128, 6], F32, tag="sg")
                tg = tp.tile([128, 2], F32, tag="tg")
                nc.scalar.activation(sg[:, 0:6], psum[:, 0:6], AFT.Sigmoid)
                nc.scalar.activation(tg[:, 0:2], psum[:, 6:8], AFT.Tanh)
                u = tp.tile([128, 2], F32, tag="u")
                v = tp.tile([128, 2], F32, tag="v")
                nc.vector.tensor_mul(u[:, 0:2], sg[:, 0:4:3], tg[:, 0:2])
                nc.vector.tensor_mul(v[:, 0:2], sg[:, 1:5:3], c_sel[:, 0:2])
                nc.vector.tensor_add(c[:, 0:2], u[:, 0:2], v[:, 0:2])
                tc_t = tp.tile([128, 2], F32, tag="tc")
                nc.scalar.activation(tc_t[:, 0:2], c[:, 0:2], AFT.Tanh)
                nc.vector.tensor_mul(h32[:, 0:2], sg[:, 2:6:3], tc_t[:, 0:2])
                hb = tp.tile([128, 2], BF16, tag="hb")
                nc.vector.tensor_copy(hb[:, 0:2], h32[:, 0:2])
                nc.sync.dma_start(ag_in[:], hb[:])
                nc.gpsimd.collective_compute(
                    "AllGather", mybir.AluOpType.bypass,
                    replica_groups=[list(range(NCORES))],
                    ins=[ag_in.opt()], outs=[ag_out.opt()],
                )
                nc.sync.dma_start(
                    h_all[:].rearrange("p (m j) -> p m j", m=NCORES),
                    ag_out[:].rearrange("(m p) j -> p m j", m=NCORES))

            nc.sync.dma_start(hall_out[:], h_all[:])
            nc.sync.dma_start(c_out[:], c[:])
            nc.sync.dma_start(h32_out[:], h32[:])
    nc.compile()
    return nc


class _SegRunner:
    """Jit a compiled bass segment for repeated multi-core execution."""

    def __init__(self, nc):
        import jax
        import jax.numpy as jnp
        from jax.experimental.shard_map import shard_map
        from jax.sharding import Mesh, PartitionSpec
        import concourse.mybir as mybir
        from concourse import bass2jax
        bass2jax.install_neuronx_cc_hook()
        self.jax = jax
        partition_name = nc.partition_id_tensor.name if nc.partition_id_tensor else None
        in_names, out_names, out_avals, zero_shapes = [], [], [], []
        for alloc in nc.m.functions[0].allocations:
            if not isinstance(alloc, mybir.MemoryLocationSet):
                continue
            name = alloc.memorylocations[0].name
            if alloc.kind == "ExternalInput":
                if name != partition_name:
                    in_names.append(name)
            elif alloc.kind == "ExternalOutput":
                out_names.append(name)
                shape = tuple(alloc.tensor_shape)
                dtype = mybir.dt.np(alloc.dtype)
                out_avals.append(jax.core.ShapedArray(shape, dtype))
                zero_shapes.append((shape, dtype))
        self.in_names, self.out_names = in_names, out_names
        self.zero_shapes = zero_shapes
        n_params, n_outs = len(in_names), len(out_names)

        def _body(*args):
            operands = list(args)
            if partition_name is not None:
                operands.append(bass2jax.partition_id_tensor())
            names = list(in_names) + list(out_names) + (
                [partition_name] if partition_name else [])
            outs = bass2jax._bass_exec_p.bind(
                *operands,
                out_avals=tuple(out_avals),
                in_names=tuple(names),
                out_names=tuple(out_names),
                lowering_input_output_aliases=(),
                sim_require_finite=True,
                sim_require_nnan=True,
                nc=nc,
            )
            return tuple(outs)

        devices = jax.devices()[:NCORES]
        mesh = Mesh(np.asarray(devices), ("core",))
        in_specs = (PartitionSpec("core"),) * (n_params + n_outs)
        out_specs = (PartitionSpec("core"),) * n_outs
        self.fn = jax.jit(
            shard_map(_body, mesh=mesh, in_specs=in_specs,
                      out_specs=out_specs, check_rep=False),
            donate_argnums=tuple(range(n_params, n_params + n_outs)),
            keep_unused=True,
        )

    def __call__(self, named_inputs):
        args = [named_inputs[nm] for nm in self.in_names]
        zeros = [np.zeros((NCORES * sh[0], *sh[1:]), dt)
                 for sh, dt in self.zero_shapes]
        outs = self.fn(*args, *zeros)
        return dict(zip(self.out_names, outs))


def _concat_cores(arrs):
    return np.concatenate(arrs, axis=0)


def kernel(**inputs) -> np.ndarray:
    import jax
    S, Wt_cores, PS_cores = _host_prep(**inputs)

    nseg = S // SEG_D
    rem = S - nseg * SEG_D

    runners = {}
    if nseg:
        runners[SEG_D] = _SegRunner(_build_segment(SEG_D))
    if rem:
        runners[rem] = _SegRunner(_build_segment(rem))

    wt_dev = jax.device_put(_concat_cores(Wt_cores))
    ps_slices = []
    pos = 0
    plan = [SEG_D] * nseg + ([rem] if rem else [])
    for d in plan:
        ps_slices.append(jax.device_put(
            _concat_cores([PS_cores[m][pos:pos + d] for m in range(NCORES)])))
        pos += d

    def run_chain():
        import time as _time
        hall = np.zeros((NCORES * 128, 16), ml_dtypes.bfloat16)
        cst = np.zeros((NCORES * 128, 2), np.float32)
        outs = None
        t0 = _time.perf_counter()
        for d, ps_dev in zip(plan, ps_slices):
            outs = runners[d](dict(wt=wt_dev, perstep=ps_dev,
                                   hall_in=hall, c_in=cst))
            hall = outs["hall_out"]
            cst = outs["c_out"]
        res = np.asarray(outs["h32_out"])
        dt = _time.perf_counter() - t0
        return res, dt

    _, _warm_dt = run_chain()          # compile + warm
    h32_flat, timed_dt = run_chain()   # timed pass
    KERNEL_STATS["exec_time_ns"] = int(timed_dt * 1e9)
    KERNEL_STATS["warm_wall_s"] = _warm_dt
    h32 = h32_flat.reshape(NCORES, 128, 2)

    h = np.zeros(H, np.float32)
    for q in range(NCORES):
        h[128 * q:128 * (q + 1)] = h32[q][:, 0]
        h[1024 + 128 * q:1024 + 128 * (q + 1)] = h32[q][:, 1]
    return h.reshape(1, 1, H)


# revision 6
# speedup vs baseline: 1.5475x; 1.5475x over previous
"""Trainium2 Bass kernel for nn_AwkwardRNNDoubleJagged (8-core tensor-parallel LSTM).

Strategy
--------
The module is one long, strictly sequential LSTM chain: 64 particles, each a
ragged sequence of scalar inputs, with the event-level half-state carried
across particles.  Only sum(lengths) steps actually change state, so the host
flattens the valid steps into one schedule.

Per step the dominant work is the matvec W_hh @ h with W_hh [8192, 2048].
We shard the 4H gate dimension across the 8 NeuronCores (1024 rows/core,
bf16, SBUF-resident) and all-gather the bf16 hidden state (256 floats/core)
between steps via the ncfw AllGather collective.

Particle boundaries (h,c <- [second_half, 0]) are expressed as per-step mask
data (m=boundary, w=1-m), so every step runs the identical instruction
sequence and a single compiled program can execute any run of D consecutive
steps.  Since long chains of collectives in one NEFF are not reliable, the
chain is split into segments of D=64 steps executed back-to-back (state
stays device-resident between segments).

Hidden layout: h_all[p, 2q+e] = h[e*1024 + 128 q + p]; core m owns q = m.
Gate columns: [i0, f0, o0, i1, f1, o1, g0, g1] (Xe = gate X, hidden half e).
"""
import numpy as np
import ml_dtypes

NCORES = 8
H = 2048
SEG_D = 64
KERNEL_STATS = {}
GATE_OF_COL = [0, 1, 3, 0, 1, 3, 2, 2]
HALF_OF_COL = [0, 0, 0, 1, 1, 1, 0, 1]


def _host_prep(event, lengths, W_ih, W_hh, b_ih, b_hh):
    event = np.asarray(event, np.float32)
    lengths = np.asarray(lengths).astype(np.int64)
    W_hh = np.asarray(W_hh, np.float32)
    w_in = np.asarray(W_ih, np.float32)[:, 0]
    bsum = np.asarray(b_ih, np.float32) + np.asarray(b_hh, np.float32)

    xs, bnd = [], []
    for p in range(event.shape[0]):
        for t in range(int(lengths[p])):
            xs.append(event[p, t])
            bnd.append(1.0 if t == 0 else 0.0)
    xs = np.asarray(xs, np.float32)
    bnd = np.asarray(bnd, np.float32)
    S = len(xs)

    cols = np.arange(8)
    gates = np.asarray(GATE_OF_COL)[cols]
    halves = np.asarray(HALF_OF_COL)[cols]
    p_idx = np.arange(128)
    m_idx = np.arange(NCORES)
    rows = (gates[None, :, None] * 2048 + halves[None, :, None] * 1024
            + 128 * m_idx[:, None, None] + p_idx[None, None, :])  # [m, col, p]
    kc = np.arange(16)
    qs, es = kc // 2, kc % 2
    khid = es[:, None] * 1024 + 128 * qs[:, None] + np.arange(128)[None, :]

    Wt_cores, PS_cores = [], []
    for m in range(NCORES):
        g = W_hh[rows[m][:, None, None, :], khid[None, :, :, None]]
        g = np.transpose(g, (2, 0, 1, 3)).reshape(128, 8 * 16 * 128)
        Wt_cores.append(np.ascontiguousarray(g.astype(ml_dtypes.bfloat16)))
        r = rows[m]
        Bt = bsum[r][None] + w_in[r][None] * xs[:, None, None]   # [S, 8, 128]
        ps = np.zeros((S, 128, 10), np.float32)
        ps[:, :, 0:8] = np.transpose(Bt, (0, 2, 1))
        ps[:, :, 8] = bnd[:, None]
        ps[:, :, 9] = 1.0 - bnd[:, None]
        PS_cores.append(np.ascontiguousarray(ps))
    return S, Wt_cores, PS_cores


def _patch_birsim_off():
    """walrus's birsim pass simulates the whole program at compile time;
    for our ~10k-instruction segments that is minutes of compile for no
    benefit.  Rebuild bir_verify_and_optimise with birsim disabled."""
    import inspect
    import concourse.bass_utils as bu
    if getattr(bu, "_birsim_patched", False):
        return
    src = inspect.getsource(bu.bir_verify_and_optimise)
    src = src.replace('"--enable-birsim=true",', '"--enable-birsim=false",')
    exec(src, bu.__dict__)
    bu._birsim_patched = True


def _build_segment(D):
    import concourse.bass as bass
    import concourse.bacc as bacc
    import concourse.tile as tile
    import concourse.mybir as mybir
    _patch_birsim_off()
    F32 = mybir.dt.float32
    BF16 = mybir.dt.bfloat16
    AFT = mybir.ActivationFunctionType

    nc = bacc.Bacc("TRN2", target_bir_lowering=False, debug=False,
                   num_devices=NCORES)
    wt_dram = nc.dram_tensor("wt", [128, 8 * 16 * 128], BF16, kind="ExternalInput")
    ps_dram = nc.dram_tensor("perstep", [D, 128, 10], F32, kind="ExternalInput")
    hall_in = nc.dram_tensor("hall_in", [128, 16], BF16, kind="ExternalInput")
    c_in = nc.dram_tensor("c_in", [128, 2], F32, kind="ExternalInput")
    hall_out = nc.dram_tensor("hall_out", [128, 16], BF16, kind="ExternalOutput")
    c_out = nc.dram_tensor("c_out", [128, 2], F32, kind="ExternalOutput")
    h32_out = nc.dram_tensor("h32_out", [128, 2], F32, kind="ExternalOutput")

    with tile.TileContext(nc) as tc:
        with tc.tile_pool(name="wt", bufs=1) as wtp, \
             tc.tile_pool(name="state", bufs=1) as stp, \
             tc.tile_pool(name="psin", bufs=4) as psp_in, \
             tc.tile_pool(name="tmp", bufs=3) as tp, \
             tc.tile_pool(name="gps", bufs=2, space="PSUM") as psp, \
             tc.tile_pool(name="dram", bufs=2, space="DRAM") as dr:

            wt = wtp.tile([128, 8 * 16 * 128], BF16)
            nc.sync.dma_start(wt[:], wt_dram[:])
            h_all = stp.tile([128, 16], BF16)
            c = stp.tile([128, 2], F32)
            h32 = stp.tile([128, 2], F32)
            nc.sync.dma_start(h_all[:], hall_in[:])
            nc.sync.dma_start(c[:], c_in[:])

            ag_in = dr.tile([128, 2], BF16, tag="agin")
            ag_out = dr.tile([128 * NCORES, 2], BF16, tag="agout")

            def wtile(col, kcc):
                return wt[:, bass.ts(col * 16 + kcc, 128)]

            for s in range(D):
                ps = psp_in.tile([128, 10], F32, tag="ps")
                nc.sync.dma_start(ps[:], ps_dram[s])
                mm = ps[:, 8:9]
                ww = ps[:, 9:10]

                h_use = tp.tile([128, 16], BF16, tag="huse")
                nc.vector.tensor_scalar_mul(h_use[:], h_all[:], ww)
                nc.vector.scalar_tensor_tensor(
                    h_use[:, 0:16:2], h_all[:, 1:16:2], mm, h_use[:, 0:16:2],
                    op0=mybir.AluOpType.mult, op1=mybir.AluOpType.add)
                c_sel = tp.tile([128, 2], F32, tag="csel")
                nc.vector.tensor_scalar_mul(c_sel[:], c[:], ww)
                nc.vector.scalar_tensor_tensor(
                    c_sel[:, 0:1], c[:, 1:2], mm, c_sel[:, 0:1],
                    op0=mybir.AluOpType.mult, op1=mybir.AluOpType.add)

                psum = psp.tile([128, 8], F32, tag="gates")
                for col in range(8):
                    o = psum[:, col:col + 1]
                    for kcc in range(16):
                        nc.tensor.matmul(o, wtile(col, kcc),
                                         h_use[:, kcc:kcc + 1],
                                         start=(kcc == 0), stop=(kcc == 15))
                nc.vector.tensor_add(psum[:, 0:8], psum[:, 0:8], ps[:, 0:8])
                sg = tp.tile([128, 6], F32, tag="sg")
                tg = tp.tile([128, 2], F32, tag="tg")
                nc.scalar.activation(sg[:, 0:6], psum[:, 0:6], AFT.Sigmoid)
                nc.scalar.activation(tg[:, 0:2], psum[:, 6:8], AFT.Tanh)
                u = tp.tile([128, 2], F32, tag="u")
                v = tp.tile([128, 2], F32, tag="v")
                nc.vector.tensor_mul(u[:, 0:2], sg[:, 0:4:3], tg[:, 0:2])
                nc.vector.tensor_mul(v[:, 0:2], sg[:, 1:5:3], c_sel[:, 0:2])
                nc.vector.tensor_add(c[:, 0:2], u[:, 0:2], v[:, 0:2])
                tc_t = tp.tile([128, 2], F32, tag="tc")
                nc.scalar.activation(tc_t[:, 0:2], c[:, 0:2], AFT.Tanh)
                nc.vector.tensor_mul(h32[:, 0:2], sg[:, 2:6:3], tc_t[:, 0:2])
                hb = tp.tile([128, 2], BF16, tag="hb")
                nc.vector.tensor_copy(hb[:, 0:2], h32[:, 0:2])
                nc.sync.dma_start(ag_in[:], hb[:])
                nc.gpsimd.collective_compute(
                    "AllGather", mybir.AluOpType.bypass,
                    replica_groups=[list(range(NCORES))],
                    ins=[ag_in.opt()], outs=[ag_out.opt()],
                )
                nc.sync.dma_start(
                    h_all[:].rearrange("p (m j) -> p m j", m=NCORES),
                    ag_out[:].rearrange("(m p) j -> p m j", m=NCORES))

            nc.sync.dma_start(hall_out[:], h_all[:])
            nc.sync.dma_start(c_out[:], c[:])
            nc.sync.dma_start(h32_out[:], h32[:])
    nc.compile()
    return nc


class _SegRunner:
    """Jit a compiled bass segment for repeated multi-core execution."""

    def __init__(self, nc):
        import jax
        import jax.numpy as jnp
        from jax.experimental.shard_map import shard_map
        from jax.sharding import Mesh, PartitionSpec
        import concourse.mybir as mybir
        from concourse import bass2jax
        bass2jax.install_neuronx_cc_hook()
        self.jax = jax
        partition_name = nc.partition_id_tensor.name if nc.partition_id_tensor else None
        in_names, out_names, out_avals, zero_shapes = [], [], [], []
        for alloc in nc.m.functions[0].allocations:
            if not isinstance(alloc, mybir.MemoryLocationSet):
                continue
            name = alloc.memorylocations[0].name
            if alloc.kind == "ExternalInput":
                if name != partition_name:
                    in_names.append(name)
            elif alloc.kind == "ExternalOutput":
                out_names.append(name)
                shape = tuple(alloc.tensor_shape)
                dtype = mybir.dt.np(alloc.dtype)
                out_avals.append(jax.core.ShapedArray(shape, dtype))
                zero_shapes.append((shape, dtype))
        self.in_names, self.out_names = in_names, out_names
        self.zero_shapes = zero_shapes
        n_params, n_outs = len(in_names), len(out_names)

        def _body(*args):
            operands = list(args)
            if partition_name is not None:
                operands.append(bass2jax.partition_id_tensor())
            names = list(in_names) + list(out_names) + (
                [partition_name] if partition_name else [])
            outs = bass2jax._bass_exec_p.bind(
                *operands,
                out_avals=tuple(out_avals),
                in_names=tuple(names),
                out_names=tuple(out_names),
                lowering_input_output_aliases=(),
                sim_require_finite=True,
                sim_require_nnan=True,
                nc=nc,
            )
            return tuple(outs)

        devices = jax.devices()[:NCORES]
        mesh = Mesh(np.asarray(devices), ("core",))
        in_specs = (PartitionSpec("core"),) * (n_params + n_outs)
        out_specs = (PartitionSpec("core"),) * n_outs
        self.fn = jax.jit(
            shard_map(_body, mesh=mesh, in_specs=in_specs,
                      out_specs=out_specs, check_rep=False),
            donate_argnums=tuple(range(n_params, n_params + n_outs)),
            keep_unused=True,
        )

    def __call__(self, named_inputs):
        args = [named_inputs[nm] for nm in self.in_names]
        zeros = [np.zeros((NCORES * sh[0], *sh[1:]), dt)
                 for sh, dt in self.zero_shapes]
        outs = self.fn(*args, *zeros)
        return dict(zip(self.out_names, outs))


def _concat_cores(arrs):
    return np.concatenate(arrs, axis=0)


def kernel(**inputs) -> np.ndarray:
    import jax
    S, Wt_cores, PS_cores = _host_prep(**inputs)

    # One program for the whole chain: deep collective chains are fine on
    # this runtime, and each extra dispatch costs a full host round-trip.
    seg_d = S
    nseg = S // seg_d
    rem = S - nseg * seg_d

    runners = {}
    if nseg:
        runners[seg_d] = _SegRunner(_build_segment(seg_d))
    if rem:
        runners[rem] = _SegRunner(_build_segment(rem))

    wt_dev = jax.device_put(_concat_cores(Wt_cores))
    ps_slices = []
    pos = 0
    plan = [seg_d] * nseg + ([rem] if rem else [])
    for d in plan:
        ps_slices.append(jax.device_put(
            _concat_cores([PS_cores[m][pos:pos + d] for m in range(NCORES)])))
        pos += d

    def run_chain():
        import time as _time
        hall = np.zeros((NCORES * 128, 16), ml_dtypes.bfloat16)
        cst = np.zeros((NCORES * 128, 2), np.float32)
        outs = None
        t0 = _time.perf_counter()
        for d, ps_dev in zip(plan, ps_slices):
            outs = runners[d](dict(wt=wt_dev, perstep=ps_dev,
                                   hall_in=hall, c_in=cst))
            hall = outs["hall_out"]
            cst = outs["c_out"]
        res = np.asarray(outs["h32_out"])
        dt = _time.perf_counter() - t0
        return res, dt

    _, _warm_dt = run_chain()          # compile + warm
    h32_flat, timed_dt = run_chain()   # timed pass
    KERNEL_STATS["exec_time_ns"] = int(timed_dt * 1e9)
    KERNEL_STATS["warm_wall_s"] = _warm_dt
    h32 = h32_flat.reshape(NCORES, 128, 2)

    h = np.zeros(H, np.float32)
    for q in range(NCORES):
        h[128 * q:128 * (q + 1)] = h32[q][:, 0]
        h[1024 + 128 * q:1024 + 128 * (q + 1)] = h32[q][:, 1]
    return h.reshape(1, 1, H)


# revision 8
# speedup vs baseline: 1.8880x; 1.2200x over previous
"""Trainium2 Bass kernel for nn_AwkwardRNNDoubleJagged (8-core tensor-parallel LSTM).

Strategy
--------
The module is one long, strictly sequential LSTM chain: 64 particles, each a
ragged sequence of scalar inputs, with the event-level half-state carried
across particles.  Only sum(lengths) steps actually change state, so the host
flattens the valid steps into one schedule.

Per step the dominant work is the matvec W_hh @ h with W_hh [8192, 2048].
We shard the 4H gate dimension across the 8 NeuronCores (1024 rows/core,
bf16, SBUF-resident) and all-gather the bf16 hidden state (256 floats/core)
between steps via the ncfw AllGather collective.

Particle boundaries (h,c <- [second_half, 0]) are expressed as per-step mask
data (m=boundary, w=1-m), so every step runs the identical instruction
sequence.  The whole chain (all sum(lengths) steps) is compiled into a single
program / NEFF and dispatched once; the segment machinery below also supports
splitting the chain if a shorter program is ever needed.

Hidden layout: h_all[p, 2q+e] = h[e*1024 + 128 q + p]; core m owns q = m.
Gate columns: [i0, f0, o0, i1, f1, o1, g0, g1] (Xe = gate X, hidden half e).
"""
import numpy as np
import ml_dtypes

NCORES = 8
H = 2048
SEG_D = 64
KERNEL_STATS = {}
GATE_OF_COL = [0, 1, 3, 0, 1, 3, 2, 2]
HALF_OF_COL = [0, 0, 0, 1, 1, 1, 0, 1]


def _host_prep(event, lengths, W_ih, W_hh, b_ih, b_hh):
    event = np.asarray(event, np.float32)
    lengths = np.asarray(lengths).astype(np.int64)
    W_hh = np.asarray(W_hh, np.float32)
    w_in = np.asarray(W_ih, np.float32)[:, 0]
    bsum = np.asarray(b_ih, np.float32) + np.asarray(b_hh, np.float32)

    xs, bnd = [], []
    for p in range(event.shape[0]):
        for t in range(int(lengths[p])):
            xs.append(event[p, t])
            bnd.append(1.0 if t == 0 else 0.0)
    xs = np.asarray(xs, np.float32)
    bnd = np.asarray(bnd, np.float32)
    S = len(xs)

    cols = np.arange(8)
    gates = np.asarray(GATE_OF_COL)[cols]
    halves = np.asarray(HALF_OF_COL)[cols]
    p_idx = np.arange(128)
    m_idx = np.arange(NCORES)
    rows = (gates[None, :, None] * 2048 + halves[None, :, None] * 1024
            + 128 * m_idx[:, None, None] + p_idx[None, None, :])  # [m, col, p]
    kc = np.arange(16)
    qs, es = kc // 2, kc % 2
    khid = es[:, None] * 1024 + 128 * qs[:, None] + np.arange(128)[None, :]

    Wt_cores, PS_cores = [], []
    for m in range(NCORES):
        g = W_hh[rows[m][:, None, None, :], khid[None, :, :, None]]
        g = np.transpose(g, (2, 0, 1, 3)).reshape(128, 8 * 16 * 128)
        Wt_cores.append(np.ascontiguousarray(g.astype(ml_dtypes.bfloat16)))
        r = rows[m]
        Bt = bsum[r][None] + w_in[r][None] * xs[:, None, None]   # [S, 8, 128]
        ps = np.zeros((S, 128, 10), np.float32)
        ps[:, :, 0:8] = np.transpose(Bt, (0, 2, 1))
        ps[:, :, 8] = bnd[:, None]
        ps[:, :, 9] = 1.0 - bnd[:, None]
        PS_cores.append(np.ascontiguousarray(ps))
    return S, Wt_cores, PS_cores


def _patch_birsim_off():
    """walrus's birsim pass simulates the whole program at compile time;
    for our ~10k-instruction segments that is minutes of compile for no
    benefit.  Rebuild bir_verify_and_optimise with birsim disabled."""
    import inspect
    import concourse.bass_utils as bu
    if getattr(bu, "_birsim_patched", False):
        return
    try:
        src = inspect.getsource(bu.bir_verify_and_optimise)
    except OSError:
        return  # already redefined by someone else
    src = src.replace('"--enable-birsim=true",', '"--enable-birsim=false",')
    exec(src, bu.__dict__)
    bu._birsim_patched = True


def _build_segment(D):
    import concourse.bass as bass
    import concourse.bacc as bacc
    import concourse.tile as tile
    import concourse.mybir as mybir
    _patch_birsim_off()
    F32 = mybir.dt.float32
    BF16 = mybir.dt.bfloat16
    AFT = mybir.ActivationFunctionType

    nc = bacc.Bacc("TRN2", target_bir_lowering=False, debug=False,
                   num_devices=NCORES)
    wt_dram = nc.dram_tensor("wt", [128, 8 * 16 * 128], BF16, kind="ExternalInput")
    ps_dram = nc.dram_tensor("perstep", [D, 128, 10], F32, kind="ExternalInput")
    hall_in = nc.dram_tensor("hall_in", [128, 16], BF16, kind="ExternalInput")
    c_in = nc.dram_tensor("c_in", [128, 2], F32, kind="ExternalInput")
    hall_out = nc.dram_tensor("hall_out", [128, 16], BF16, kind="ExternalOutput")
    c_out = nc.dram_tensor("c_out", [128, 2], F32, kind="ExternalOutput")
    h32_out = nc.dram_tensor("h32_out", [128, 2], F32, kind="ExternalOutput")

    with tile.TileContext(nc) as tc:
        with tc.tile_pool(name="wt", bufs=1) as wtp, \
             tc.tile_pool(name="state", bufs=1) as stp, \
             tc.tile_pool(name="psin", bufs=4) as psp_in, \
             tc.tile_pool(name="tmp", bufs=3) as tp, \
             tc.tile_pool(name="gps", bufs=2, space="PSUM") as psp, \
             tc.tile_pool(name="dram", bufs=2, space="DRAM") as dr:

            wt = wtp.tile([128, 8 * 16 * 128], BF16)
            nc.sync.dma_start(wt[:], wt_dram[:])
            h_all = stp.tile([128, 16], BF16)
            c = stp.tile([128, 2], F32)
            h32 = stp.tile([128, 2], F32)
            nc.sync.dma_start(h_all[:], hall_in[:])
            nc.sync.dma_start(c[:], c_in[:])

            ag_in = dr.tile([128, 2], BF16, tag="agin")
            ag_out = dr.tile([128 * NCORES, 2], BF16, tag="agout")

            def wtile(col, kcc):
                return wt[:, bass.ts(col * 16 + kcc, 128)]

            for s in range(D):
                ps = psp_in.tile([128, 10], F32, tag="ps")
                nc.sync.dma_start(ps[:], ps_dram[s])
                mm = ps[:, 8:9]
                ww = ps[:, 9:10]

                h_use = tp.tile([128, 16], BF16, tag="huse")
                nc.vector.tensor_scalar_mul(h_use[:], h_all[:], ww)
                nc.vector.scalar_tensor_tensor(
                    h_use[:, 0:16:2], h_all[:, 1:16:2], mm, h_use[:, 0:16:2],
                    op0=mybir.AluOpType.mult, op1=mybir.AluOpType.add)
                c_sel = tp.tile([128, 2], F32, tag="csel")
                nc.vector.tensor_scalar_mul(c_sel[:], c[:], ww)
                nc.vector.scalar_tensor_tensor(
                    c_sel[:, 0:1], c[:, 1:2], mm, c_sel[:, 0:1],
                    op0=mybir.AluOpType.mult, op1=mybir.AluOpType.add)

                psum = psp.tile([128, 8], F32, tag="gates")
                for col in range(8):
                    o = psum[:, col:col + 1]
                    for kcc in range(16):
                        nc.tensor.matmul(o, wtile(col, kcc),
                                         h_use[:, kcc:kcc + 1],
                                         start=(kcc == 0), stop=(kcc == 15))
                nc.vector.tensor_add(psum[:, 0:8], psum[:, 0:8], ps[:, 0:8])
                sg = tp.tile([128, 6], F32, tag="sg")
                tg = tp.tile([128, 2], F32, tag="tg")
                nc.scalar.activation(sg[:, 0:6], psum[:, 0:6], AFT.Sigmoid)
                nc.scalar.activation(tg[:, 0:2], psum[:, 6:8], AFT.Tanh)
                u = tp.tile([128, 2], F32, tag="u")
                v = tp.tile([128, 2], F32, tag="v")
                nc.vector.tensor_mul(u[:, 0:2], sg[:, 0:4:3], tg[:, 0:2])
                nc.vector.tensor_mul(v[:, 0:2], sg[:, 1:5:3], c_sel[:, 0:2])
                nc.vector.tensor_add(c[:, 0:2], u[:, 0:2], v[:, 0:2])
                tc_t = tp.tile([128, 2], F32, tag="tc")
                nc.scalar.activation(tc_t[:, 0:2], c[:, 0:2], AFT.Tanh)
                nc.vector.tensor_mul(h32[:, 0:2], sg[:, 2:6:3], tc_t[:, 0:2])
                hb = tp.tile([128, 2], BF16, tag="hb")
                nc.vector.tensor_copy(hb[:, 0:2], h32[:, 0:2])
                nc.sync.dma_start(ag_in[:], hb[:])
                nc.gpsimd.collective_compute(
                    "AllGather", mybir.AluOpType.bypass,
                    replica_groups=[list(range(NCORES))],
                    ins=[ag_in.opt()], outs=[ag_out.opt()],
                )
                nc.sync.dma_start(
                    h_all[:].rearrange("p (m j) -> p m j", m=NCORES),
                    ag_out[:].rearrange("(m p) j -> p m j", m=NCORES))

            nc.sync.dma_start(hall_out[:], h_all[:])
            nc.sync.dma_start(c_out[:], c[:])
            nc.sync.dma_start(h32_out[:], h32[:])
    nc.compile()
    return nc


class _SegRunner:
    """Jit a compiled bass segment for repeated multi-core execution."""

    def __init__(self, nc):
        import jax
        import jax.numpy as jnp
        from jax.experimental.shard_map import shard_map
        from jax.sharding import Mesh, PartitionSpec
        import concourse.mybir as mybir
        from concourse import bass2jax
        bass2jax.install_neuronx_cc_hook()
        self.jax = jax
        partition_name = nc.partition_id_tensor.name if nc.partition_id_tensor else None
        in_names, out_names, out_avals, zero_shapes = [], [], [], []
        for alloc in nc.m.functions[0].allocations:
            if not isinstance(alloc, mybir.MemoryLocationSet):
                continue
            name = alloc.memorylocations[0].name
            if alloc.kind == "ExternalInput":
                if name != partition_name:
                    in_names.append(name)
            elif alloc.kind == "ExternalOutput":
                out_names.append(name)
                shape = tuple(alloc.tensor_shape)
                dtype = mybir.dt.np(alloc.dtype)
                out_avals.append(jax.core.ShapedArray(shape, dtype))
                zero_shapes.append((shape, dtype))
        self.in_names, self.out_names = in_names, out_names
        self.zero_shapes = zero_shapes
        n_params, n_outs = len(in_names), len(out_names)

        def _body(*args):
            operands = list(args)
            if partition_name is not None:
                operands.append(bass2jax.partition_id_tensor())
            names = list(in_names) + list(out_names) + (
                [partition_name] if partition_name else [])
            outs = bass2jax._bass_exec_p.bind(
                *operands,
                out_avals=tuple(out_avals),
                in_names=tuple(names),
                out_names=tuple(out_names),
                lowering_input_output_aliases=(),
                sim_require_finite=True,
                sim_require_nnan=True,
                nc=nc,
            )
            return tuple(outs)

        devices = jax.devices()[:NCORES]
        mesh = Mesh(np.asarray(devices), ("core",))
        in_specs = (PartitionSpec("core"),) * (n_params + n_outs)
        out_specs = (PartitionSpec("core"),) * n_outs
        self.fn = jax.jit(
            shard_map(_body, mesh=mesh, in_specs=in_specs,
                      out_specs=out_specs, check_rep=False),
            donate_argnums=tuple(range(n_params, n_params + n_outs)),
            keep_unused=True,
        )

    def __call__(self, named_inputs):
        args = [named_inputs[nm] for nm in self.in_names]
        zeros = [np.zeros((NCORES * sh[0], *sh[1:]), dt)
                 for sh, dt in self.zero_shapes]
        outs = self.fn(*args, *zeros)
        return dict(zip(self.out_names, outs))


def _concat_cores(arrs):
    return np.concatenate(arrs, axis=0)


def kernel(**inputs) -> np.ndarray:
    import jax
    S, Wt_cores, PS_cores = _host_prep(**inputs)

    # One program for the whole chain: deep collective chains are fine on
    # this runtime, and each extra dispatch costs a full host round-trip.
    seg_d = S
    nseg = S // seg_d
    rem = S - nseg * seg_d

    runners = {}
    if nseg:
        runners[seg_d] = _SegRunner(_build_segment(seg_d))
    if rem:
        runners[rem] = _SegRunner(_build_segment(rem))

    wt_dev = jax.device_put(_concat_cores(Wt_cores))
    ps_slices = []
    pos = 0
    plan = [seg_d] * nseg + ([rem] if rem else [])
    for d in plan:
        ps_slices.append(jax.device_put(
            _concat_cores([PS_cores[m][pos:pos + d] for m in range(NCORES)])))
        pos += d

    def run_chain():
        import time as _time
        hall = np.zeros((NCORES * 128, 16), ml_dtypes.bfloat16)
        cst = np.zeros((NCORES * 128, 2), np.float32)
        outs = None
        t0 = _time.perf_counter()
        for d, ps_dev in zip(plan, ps_slices):
            outs = runners[d](dict(wt=wt_dev, perstep=ps_dev,
                                   hall_in=hall, c_in=cst))
            hall = outs["hall_out"]
            cst = outs["c_out"]
        res = np.asarray(outs["h32_out"])
        dt = _time.perf_counter() - t0
        return res, dt

    _, _warm_dt = run_chain()          # compile + warm
    h32_flat, timed_dt = run_chain()   # timed pass
    KERNEL_STATS["exec_time_ns"] = int(timed_dt * 1e9)
    KERNEL_STATS["warm_wall_s"] = _warm_dt
    h32 = h32_flat.reshape(NCORES, 128, 2)

    h = np.zeros(H, np.float32)
    for q in range(NCORES):
        h[128 * q:128 * (q + 1)] = h32[q][:, 0]
        h[1024 + 128 * q:1024 + 128 * (q + 1)] = h32[q][:, 1]
    return h.reshape(1, 1, H)
